# revision 43
# baseline (speedup 1.0000x reference)
"""Trainium2 Bass kernel for nn_EncoderOnlyBlock (4-head full-dim encoder block).

Sharding: data-parallel, 8 cores = (batch b, seq-half). Each core computes
its 1024 query tokens end-to-end for all 4 heads. K projections are computed
for OWN tokens only and exchanged with the sibling core via a paired
AllGather (DRAM bounce buffers), pipelined for all 4 heads upfront so the
collectives hide under the Q/score chains; kt columns and the x rows feeding
the A@V chains both use the gathered batch-natural token order, keeping the
program fully symmetric across cores.

All attention/projection matmuls run in fp8e4 with DoubleRow perf mode
(2 fp8 MACs/PE-cell/cycle; lhsT/rhs sliced [128, 2, n] over contraction
pairs). Scores are computed TRANSPOSED (S^T = K^T^T Q^T per sj-block) so
exp(S^T) feeds the A@V ("M") chains directly with no PE transposes of the
attention matrix. Softmax normalization is deferred: E = exp(S/sqrt(D)) is
used unnormalized; per-query reciprocal row-sums (ones-vector DR matmuls
over E^T + tiny PE transposes into [si-partition] layout) scale each head's
contribution at the very end. Wv and W1 are fused on the host
(proj_h = A_h x (Wv_h W1_h)), eliminating the Wv chains entirely.

Per-core math (fp8 DR matmuls, fp32 PSUM accumulation):
  K^T_h = (16Wk_h)^T @ x^T / 16          [e, sj]  (bk dropped: softmax-inv.)
  Q^T_h = (16Wq_h)^T @ x^T[:, :1024]/16 + bq      [e, si]
  S^T   = K^T^T Q^T  (per sj-block)      [sj, si]
  E^T   = exp(S^T / 32)  fp8, unnormalized
  rs    = ones^T @ E^T;  rec = 1/(512*rs)         [si(part), head, si-tile]
  M     = x^T @ E^T / 4                  [d, si]  (A@V reassociation)
  proj += rec_h * (M^T @ (2048 Wv_h W1_h))   [si, f]
  cvec  = b1 + sum_h bv_h@W1_h folded into xres on host (A rows sum to 1).
  u1    = xres' + proj;  LN1 -> y (bf16);  z = y@W2 + b2;  LN2(y+z) -> out
LN means/vars via sum & sum-of-squares accumulators; g1/be1/g2/be2 applied
only when not exactly ones/zeros (checked on host). LN1 stats and the first
y transposes are interleaved into head-3's proj loop (lagged) so the
strict-FIFO engine queues pipeline across si-tiles; the z-chains + LN2 run
as a dense PE tail right after. fp8 scale bookkeeping keeps every
tensor within TRN-e4m3 range (max 240): weights x16/x2048 on host, M /4,
with the inverse folded into the activation scales and rec.
"""

import numpy as np
import ml_dtypes

BF = ml_dtypes.bfloat16
F8 = ml_dtypes.float8_e4m3  # TRN-style e4m3: max 240
P = 128
D = 1024
S = 2048
SI = 1024
H = 4
ET = D // P       # 8 e/d/f 128-blocks
SJT = S // P      # 16 sj 128-blocks
SIT = SI // P     # 8 si 128-blocks
SCALE = 1.0 / 32.0  # 1/sqrt(D)
EPS = 1e-5

_CACHE = {}


def _emit(nc, tc, A, trivial_gbe):
    """Emit the per-core program. A: dict name -> dram AP."""
    from contextlib import ExitStack

    import concourse.bass as bass
    import concourse.mybir as mybir
    from concourse.masks import make_identity

    f32 = mybir.dt.float32
    bf16 = mybir.dt.bfloat16
    f8 = mybir.dt.float8e4
    Act = mybir.ActivationFunctionType
    Alu = mybir.AluOpType
    DR = mybir.MatmulPerfMode.DoubleRow

    def dr_chain(ps, lhsT_of, rhs_of, npairs):
        """Emit a DoubleRow accumulation chain of `npairs` k-pair matmuls."""
        for kp in range(npairs):
            nc.tensor.matmul(
                ps,
                lhsT=lhsT_of(kp),
                rhs=rhs_of(kp),
                start=(kp == 0), stop=(kp == npairs - 1),
                perf_mode=DR,
            )

    with ExitStack() as ctx:
        consts = ctx.enter_context(tc.tile_pool(name="consts", bufs=1))
        psA = ctx.enter_context(tc.tile_pool(name="psA", bufs=3, space="PSUM"))
        psB = ctx.enter_context(tc.tile_pool(name="psB", bufs=1, space="PSUM"))
        proj_pool = ctx.enter_context(tc.tile_pool(name="pj", bufs=1))
        # LN-phase inputs whose DMAs we start mid-head-loop
        xr_pool = ctx.enter_context(tc.tile_pool(name="xr", bufs=2))
        w2_pool = ctx.enter_context(tc.tile_pool(name="w2", bufs=ET))
        # LN1 (phase A) pools — phase A is emitted inside head-3's proj loop
        u_pool = ctx.enter_context(tc.tile_pool(name="up", bufs=2))
        sq_pool = ctx.enter_context(tc.tile_pool(name="sq", bufs=1))
        st_pool = ctx.enter_context(tc.tile_pool(name="st", bufs=8))
        lnp = ctx.enter_context(tc.tile_pool(name="lnp", bufs=1))
        yt_pool = ctx.enter_context(tc.tile_pool(name="yt", bufs=2))
        ot_pool = ctx.enter_context(tc.tile_pool(name="ot", bufs=1))

        ident = consts.tile([P, P], bf16, tag="ident")
        make_identity(nc, ident[:])
        bqr_sb = consts.tile([P, H * ET], f32, tag="bqr")
        nc.sync.dma_start(out=bqr_sb[:], in_=A["bqr"][:])
        buv_sb = consts.tile([1, D], bf16, tag="buv")
        nc.sync.dma_start(out=buv_sb[:], in_=A["buv"][:])
        ones_sb = consts.tile([1, P], bf16, tag="ones")
        nc.vector.memset(ones_sb[:], 1.0)
        ones8_sb = consts.tile([P, 2, 16], f8, tag="ones8")
        nc.vector.memset(ones8_sb[:], 1.0)
        eps_sb = consts.tile([P, 1], f32, tag="eps")
        nc.vector.memset(eps_sb[:], EPS)
        # per-head reciprocal row-sums, [si(partition), head, si-tile]
        rsT_sb = consts.tile([P, H, SIT], f32, tag="rsT")
        if not trivial_gbe:
            gbe_sb = lnp.tile([P, 4, D], f32, tag="gbe")
            gbe_bc = bass.AP(
                tensor=A["gbe"].tensor, offset=A["gbe"].offset,
                ap=[[0, P], A["gbe"].ap[0], A["gbe"].ap[1]],
            )
            nc.gpsimd.dma_start(out=gbe_sb[:], in_=gbe_bc)
        yb_sb = lnp.tile([P, SIT, D], bf16, tag="yb")

        def ln_stats(src, rsum):
            """-> (mu, rstd) [P,1] tiles from src [P,D] + its row-sum."""
            sq = sq_pool.tile([P, D], f32, tag="sq")
            sumsq = st_pool.tile([P, 1], f32, tag="sumsq")
            nc.scalar.activation(out=sq[:], in_=src, func=Act.Square,
                                 accum_out=sumsq[:])
            mu = st_pool.tile([P, 1], f32, tag="mu")
            nc.scalar.mul(mu[:], rsum, 1.0 / D)
            # (rsum*mu - sumsq) = -D*var;  std = sqrt(-1/D * that + eps)
            nv = st_pool.tile([P, 1], f32, tag="nv")
            nc.vector.scalar_tensor_tensor(
                out=nv[:], in0=rsum, scalar=mu[:], in1=sumsq[:],
                op0=Alu.mult, op1=Alu.subtract,
            )
            rstd = st_pool.tile([P, 1], f32, tag="rstd")
            nc.scalar.activation(out=rstd[:], in_=nv[:], func=Act.Sqrt,
                                 scale=-1.0 / D, bias=eps_sb[:])
            nc.vector.reciprocal(rstd[:], rstd[:])
            return mu, rstd

        xr_tiles = [None] * SIT

        def emit_phase_a(t):
            """u1 = x + proj -> LN1 stats -> y (bf16) for si-tile t."""
            u1 = u_pool.tile([P, D], f32, tag="u", name=f"u1_{t}")
            rs1 = st_pool.tile([P, 1], f32, tag="rs")
            nc.vector.scalar_tensor_tensor(
                out=u1[:], in0=xr_tiles[t][:], scalar=1.0,
                in1=proj_sb[:, t, :], op0=Alu.mult, op1=Alu.add,
                accum_out=rs1[:],
            )
            mu1, rstd1 = ln_stats(u1[:], rs1[:])
            yt_t = yb_sb[:, t, :]
            nc.vector.tensor_scalar(
                yt_t, u1[:], scalar1=mu1[:], scalar2=rstd1[:],
                op0=Alu.subtract, op1=Alu.mult,
            )
            if not trivial_gbe:
                nc.gpsimd.tensor_mul(yt_t, yt_t, gbe_sb[:, 0, :])
                nc.gpsimd.tensor_add(yt_t, yt_t, gbe_sb[:, 1, :])

        yt_tiles = [None] * SIT
        w2_tiles = []

        def emit_transpose(t):
            yt_tile = yt_pool.tile([P, ET, P], bf16, tag="yt")
            yt_tiles[t] = yt_tile
            pb = psB.tile([P, 1024], bf16, tag="psB")
            for fb in range(ET):
                nc.tensor.transpose(
                    pb[:, fb * P:(fb + 1) * P],
                    yb_sb[:, t, fb * P:(fb + 1) * P],
                    ident[:],
                )
            nc.vector.tensor_copy(
                yt_tile[:], pb[:].rearrange("p (f c) -> p f c", c=P)
            )

        def emit_zchain(t):
            ps = psA.tile([P, 1024], f32, tag="psA")
            for nb in range(2):
                for kc in range(ET):
                    nc.tensor.matmul(
                        ps[:, nb * 512:(nb + 1) * 512],
                        lhsT=yt_tiles[t][:, kc, :],
                        rhs=w2_tiles[kc][:, nb * 512:(nb + 1) * 512],
                        start=(kc == 0), stop=False,
                    )
                nc.tensor.matmul(
                    ps[:, nb * 512:(nb + 1) * 512],
                    lhsT=ones_sb[:, :],
                    rhs=buv_sb[:, nb * 512:(nb + 1) * 512],
                    start=False, stop=True,
                )
            # u2 = y + z + bu; LN2; out
            u2 = u_pool.tile([P, D], f32, tag="u", name=f"u2_{t}")
            rs2 = st_pool.tile([P, 1], f32, tag="rs")
            nc.vector.scalar_tensor_tensor(
                out=u2[:], in0=yb_sb[:, t, :], scalar=1.0,
                in1=ps[:], op0=Alu.mult, op1=Alu.add,
                accum_out=rs2[:],
            )
            mu2, rstd2 = ln_stats(u2[:], rs2[:])
            ot = ot_pool.tile([P, D], f32, tag="ot")
            nc.vector.tensor_scalar(
                ot[:], u2[:], scalar1=mu2[:], scalar2=rstd2[:],
                op0=Alu.subtract, op1=Alu.mult,
            )
            if not trivial_gbe:
                nc.gpsimd.tensor_mul(ot[:], ot[:], gbe_sb[:, 2, :])
                nc.gpsimd.tensor_add(ot[:], ot[:], gbe_sb[:, 3, :])
            nc.sync.dma_start(out=A["out"][t * P:(t + 1) * P, :], in_=ot[:])

        head_ctx = ExitStack()
        xpool = head_ctx.enter_context(tc.tile_pool(name="xp", bufs=1))
        wqkv_pool = head_ctx.enter_context(tc.tile_pool(name="wqkv", bufs=9))
        w1_pool = head_ctx.enter_context(tc.tile_pool(name="w1", bufs=2))
        qt_pool = head_ctx.enter_context(tc.tile_pool(name="qt", bufs=2))
        kt_pool = head_ctx.enter_context(tc.tile_pool(name="kt", bufs=1))
        kown_pool = head_ctx.enter_context(tc.tile_pool(name="kown", bufs=2))
        kdram = head_ctx.enter_context(
            tc.tile_pool(name="kdram", bufs=8, space="DRAM"))
        et_pool = head_ctx.enter_context(tc.tile_pool(name="et", bufs=1))
        m_pool = head_ctx.enter_context(tc.tile_pool(name="m", bufs=1))
        rsb_pool = head_ctx.enter_context(tc.tile_pool(name="rsb", bufs=1))

        # head-0 K weights first (tiny, gate the very first chains), then x^T
        # half-chunks hs-major so the hs=0 K-chains can start after 1MB; x
        # natural after head-0's weights are queued
        wk0_tiles = []
        xt_half = [xpool.tile([P, ET, SI], f8, tag="xth0", name="xth0")]
        for c in range(ET):
            wk_c = wqkv_pool.tile([P, ET, P], f8, tag="wqkv")
            nc.sync.dma_start(out=wk_c[:], in_=A["wkb"][0, c])
            wk0_tiles.append(wk_c)
            nc.sync.dma_start(
                out=xt_half[0][:, c, :],
                in_=A["xt"][c * P:(c + 1) * P, 0:1024],
            )

        xn_sb = xpool.tile([P, SJT, D], f8, tag="xn")

        proj_sb = proj_pool.tile([P, SIT, D], bf16, tag="proj")

        # ---- K^T for OWN tokens only, all 4 heads upfront; sibling half
        # arrives via a paired AllGather (batch-natural column order)
        kgath_tiles = []
        qt_tiles = {}

        def emit_kown(h):
            if h == 0:
                wk_tiles = wk0_tiles
            else:
                wk_tiles = []
                for c in range(ET):
                    wk_c = wqkv_pool.tile([P, ET, P], f8, tag="wqkv")
                    nc.sync.dma_start(out=wk_c[:], in_=A["wkb"][h, c])
                    wk_tiles.append(wk_c)
            kown = kown_pool.tile([P, ET, SI], f8, tag="kown")
            for c in range(ET):
                wk_c = wk_tiles[c]
                ps = psA.tile([P, 1024], f32, tag="psA")
                for nb in range(2):
                    dr_chain(
                        ps[:, nb * 512:(nb + 1) * 512],
                        lambda kp, wk_c=wk_c: wk_c[:, 2 * kp:2 * kp + 2, :],
                        lambda kp, nb=nb: xt_half[0][
                            :, 2 * kp:2 * kp + 2, nb * 512:(nb + 1) * 512],
                        ET // 2,
                    )
                nc.vector.tensor_scalar_mul(kown[:, c, :], ps[:], 1.0 / 16.0)
            kstage = kdram.tile([ET, P, SI], f8, tag="kstage")
            stage_qs = [nc.scalar, nc.sync]
            for c in range(ET):
                for hf in range(2):
                    stage_qs[(2 * c + hf) % 2].dma_start(
                        out=kstage[c, :, hf * 512:(hf + 1) * 512],
                        in_=kown[:, c, hf * 512:(hf + 1) * 512],
                    )
            kgath = kdram.tile([2, ET, P, SI], f8, tag="kgath")
            nc.gpsimd.collective_compute(
                "AllGather", Alu.bypass,
                replica_groups=[[0, 1], [2, 3], [4, 5], [6, 7]],
                ins=[kstage.opt()], outs=[kgath.opt()],
            )
            kgath_tiles.append(kgath)

        def emit_q(h):
            qt_sb = qt_pool.tile([P, ET, SI], f8, tag="qt")
            qt_tiles[h] = qt_sb
            for c in range(ET):
                wq_c = wqkv_pool.tile([P, ET, P], f8, tag="wqkv")
                nc.sync.dma_start(out=wq_c[:], in_=A["wqb"][h, c])
                ps = psA.tile([P, 1024], f32, tag="psA")
                for nb in range(2):
                    dr_chain(
                        ps[:, nb * 512:(nb + 1) * 512],
                        lambda kp: wq_c[:, 2 * kp:2 * kp + 2, :],
                        lambda kp, nb=nb: xt_half[0][
                            :, 2 * kp:2 * kp + 2, nb * 512:(nb + 1) * 512],
                        ET // 2,
                    )
                nc.scalar.activation(
                    out=qt_sb[:, c, :], in_=ps[:], func=Act.Identity,
                    scale=1.0 / 16.0,
                    bias=bqr_sb[:, h * ET + c:h * ET + c + 1],
                )

        for h in range(H):
            emit_kown(h)

        for h in range(H):
            # ---- kt readback: [e, sj] over all 2048 gathered tokens
            kt_sb = kt_pool.tile([P, ET, S], f8, tag="kt")
            kgath = kgath_tiles[h]
            for c in range(ET):
                for r in range(2):
                    nc.gpsimd.dma_start(
                        out=kt_sb[:, c, r * 1024:(r + 1) * 1024],
                        in_=kgath[r, c],
                    )

            # ---- Q^T emission hoisted one head ahead (PE runway for the
            # AllGather of head h to complete)
            if h == 0:
                emit_q(0)
            if h + 1 < H:
                emit_q(h + 1)
            qt_sb = qt_tiles[h]

            if h == 0:
                for j in range(SJT):
                    nc.sync.dma_start(out=xn_sb[:, j, :], in_=A["xn"][j * P:(j + 1) * P, :])
            if h == 2:
                # stage the LN-phase inputs; xr via the idle gpsimd queue so a
                # pool-slot wait can't stall head-3 weight DMAs on sync
                for t in range(SIT):
                    xr = xr_pool.tile([P, D], f32, tag="xr", name=f"xr{t}")
                    nc.gpsimd.dma_start(out=xr[:], in_=A["xres"][t * P:(t + 1) * P, :])
                    xr_tiles[t] = xr
                for kc in range(ET):
                    w2_kc = w2_pool.tile([P, D], bf16, tag="w2", name=f"w2{kc}")
                    nc.sync.dma_start(out=w2_kc[:], in_=A["w2"][kc * P:(kc + 1) * P, :])
                    w2_tiles.append(w2_kc)

            # ---- S^T = K^T^T Q^T per sj-block; E^T = exp(S^T/32) fp8
            et_sb = et_pool.tile([P, SJT, SI], f8, tag="et")
            for j in range(SJT):
                ps = psA.tile([P, 1024], f32, tag="psA")
                for half in range(2):
                    dr_chain(
                        ps[:, half * 512:(half + 1) * 512],
                        lambda kp, j=j: kt_sb[:, 2 * kp:2 * kp + 2, j * P:(j + 1) * P],
                        lambda kp, half=half: qt_sb[
                            :, 2 * kp:2 * kp + 2, half * 512:(half + 1) * 512],
                        ET // 2,
                    )
                nc.scalar.activation(
                    out=et_sb[:, j, :], in_=ps[:], func=Act.Exp, scale=SCALE,
                )

            # ---- row-sums: rs[si] = sum_j E^T[j, si]; rec = 1/(8*rs) [si-part]
            rs_ps = psB.tile([1, 1024], f32, tag="psB")
            for jp in range(SJT // 2):
                for half in range(2):
                    nc.tensor.matmul(
                        rs_ps[0:1, half * 512:(half + 1) * 512],
                        lhsT=ones8_sb[:, 0:2, 0:1],
                        rhs=et_sb[:, 2 * jp:2 * jp + 2, half * 512:(half + 1) * 512],
                        start=(jp == 0), stop=(jp == SJT // 2 - 1),
                        perf_mode=DR,
                    )
            rs_bf = rsb_pool.tile([1, 1024], bf16, tag="rsb")
            nc.vector.tensor_copy(rs_bf[:], rs_ps[:])

            # ---- M = x^T @ E^T : [d, si]   (m = psum/4, fp8); the rec
            # transposes are emitted after M dc=0 so the PE doesn't wait on
            # the rs_bf copy (psB bufs=1)
            m_sb = m_pool.tile([P, ET, SI], f8, tag="m")
            for dc in range(ET):
                ps = psA.tile([P, 1024], f32, tag="psA")
                for half in range(2):
                    dr_chain(
                        ps[:, half * 512:(half + 1) * 512],
                        lambda jp, dc=dc: xn_sb[:, 2 * jp:2 * jp + 2, dc * P:(dc + 1) * P],
                        lambda jp, half=half: et_sb[
                            :, 2 * jp:2 * jp + 2, half * 512:(half + 1) * 512],
                        SJT // 2,
                    )
                nc.vector.tensor_scalar_mul(m_sb[:, dc, :], ps[:], 0.25)
                if dc == 0:
                    pb = psB.tile([P, 1024], bf16, tag="psB")
                    for t in range(SIT):
                        nc.tensor.transpose(
                            pb[:, 4 * t:4 * t + 1],
                            rs_bf[0:1, t * P:(t + 1) * P],
                            ident[0:1, 0:1],
                        )
                    pb_str = pb[:, 0:4 * SIT].rearrange("p (t c) -> p t c", c=4)[:, :, 0:1]
                    nc.vector.tensor_scalar_mul(rsT_sb[:, h, :], pb_str, 512.0)
                    nc.vector.reciprocal(rsT_sb[:, h, :], rsT_sb[:, h, :])

            # ---- proj += rec_h * (M_h^T @ (Wv_h W1_h))  (psum = 512*headU@W1)
            w1_h = w1_pool.tile([P, ET, D], f8, tag="w1")
            nc.sync.dma_start(out=w1_h[:], in_=A["w1"][h])
            for t in range(SIT):
                ps = psA.tile([P, 1024], f32, tag="psA")
                for nb in range(2):
                    dr_chain(
                        ps[:, nb * 512:(nb + 1) * 512],
                        lambda ep, t=t: m_sb[:, 2 * ep:2 * ep + 2, t * P:(t + 1) * P],
                        lambda ep, nb=nb: w1_h[
                            :, 2 * ep:2 * ep + 2, nb * 512:(nb + 1) * 512],
                        ET // 2,
                    )
                if h == 0:
                    nc.vector.tensor_scalar_mul(
                        proj_sb[:, t, :], ps[:], rsT_sb[:, 0, t:t + 1]
                    )
                else:
                    nc.vector.scalar_tensor_tensor(
                        out=proj_sb[:, t, :], in0=ps[:],
                        scalar=rsT_sb[:, h, t:t + 1],
                        in1=proj_sb[:, t, :], op0=Alu.mult, op1=Alu.add,
                    )
                if h == H - 1:
                    emit_phase_a(t)
                    if t >= 4:
                        emit_transpose(t - 4)

        head_ctx.close()

        # ====== FFN2 -> LN2 tail: remaining transposes, then dense z-chains
        emit_transpose(SIT - 4)
        emit_transpose(SIT - 3)
        emit_zchain(0)
        emit_transpose(SIT - 2)
        emit_zchain(1)
        emit_transpose(SIT - 1)
        for t in range(2, SIT):
            emit_zchain(t)


def _build(trivial_gbe):
    import concourse.bass as bass
    import concourse.mybir as mybir
    import concourse.tile as tile
    from concourse import bacc

    f32 = mybir.dt.float32
    bf16 = mybir.dt.bfloat16
    f8 = mybir.dt.float8e4

    nc = bacc.Bacc("TRN2", target_bir_lowering=False, debug=False, num_devices=8)
    A = {}

    def din(name, shape, dt):
        A[name] = nc.dram_tensor(name, shape, dt, kind="ExternalInput").ap()

    din("xt", [D, S], f8)
    din("xn", [S, D], f8)
    din("xres", [SI, D], f32)
    din("wqb", [H, ET, P, ET, P], f8)
    din("wkb", [H, ET, P, ET, P], f8)
    din("w1", [H, P, ET, D], f8)
    din("w2", [D, D], bf16)
    din("bqr", [P, H * ET], f32)
    din("buv", [1, D], bf16)
    if not trivial_gbe:
        din("gbe", [4, D], f32)
    A["out"] = nc.dram_tensor("out", [SI, D], f32, kind="ExternalOutput").ap()

    with tile.TileContext(nc) as tc:
        _emit(nc, tc, A, trivial_gbe)
    nc.compile()
    return nc


def _get_nc(trivial_gbe=True):
    key = ("nc", trivial_gbe)
    if key not in _CACHE:
        _CACHE[key] = _build(trivial_gbe)
    return _CACHE[key]


def _f8(a):
    return np.clip(a, -240.0, 240.0).astype(F8)


def _prep_inputs(inputs):
    x = np.ascontiguousarray(inputs["embedding_matrix"], dtype=np.float32)
    Wq = np.asarray(inputs["Wq"], np.float32)
    bq = np.asarray(inputs["bq"], np.float32)
    Wv = np.asarray(inputs["Wv"], np.float32)
    bv = np.asarray(inputs["bv"], np.float32)
    Wk = np.asarray(inputs["Wk"], np.float32)
    W1 = np.asarray(inputs["W1"], np.float32)
    b1 = np.asarray(inputs["b1"], np.float32)
    W2 = np.asarray(inputs["W2"], np.float32)
    b2 = np.asarray(inputs["b2"], np.float32)
    g1 = np.asarray(inputs["g1"], np.float32)
    be1 = np.asarray(inputs["be1"], np.float32)
    g2 = np.asarray(inputs["g2"], np.float32)
    be2 = np.asarray(inputs["be2"], np.float32)

    trivial = (
        np.array_equal(g1, np.ones(D, np.float32))
        and np.array_equal(g2, np.ones(D, np.float32))
        and np.array_equal(be1, np.zeros(D, np.float32))
        and np.array_equal(be2, np.zeros(D, np.float32))
    )

    def pack_w(W, s):  # [H, D, D] -> [H, ET(c), P, ET(kc), P] lhsT blocks
        return np.ascontiguousarray(
            _f8((W * s).reshape(H, ET, P, ET, P).transpose(0, 3, 2, 1, 4))
        )

    wqb = pack_w(Wq, 16.0)
    wkb = pack_w(Wk, 16.0)
    # Wv@W1 fused on host: proj_h = A_h x (Wv_h W1_h).
    # [H, D, D] -> [H, P, ET, D] so one DMA per head gives [P, ET, D]
    wvw1 = np.stack([Wv[h] @ W1[h * D:(h + 1) * D] for h in range(H)])
    w1b = np.ascontiguousarray(
        _f8((wvw1 * 2048.0).reshape(H, ET, P, D).transpose(0, 2, 1, 3))
    )
    w2b = np.ascontiguousarray(W2.astype(BF))
    # bq rearranged so bias for (h, e-block c) is column h*ET+c: [P, H*ET]
    bqr = np.ascontiguousarray(bq.reshape(H, ET, P).transpose(2, 0, 1).reshape(P, H * ET))
    cvec = (b1 + sum(bv[h] @ W1[h * D:(h + 1) * D] for h in range(H)))
    buv = np.ascontiguousarray(b2.reshape(1, D).astype(BF))

    shared = {
        "wqb": wqb, "wkb": wkb, "w1": w1b, "w2": w2b,
        "bqr": bqr, "buv": buv,
    }
    if not trivial:
        shared["gbe"] = np.ascontiguousarray(np.stack([g1, be1, g2, be2]))
    in_maps = []
    for core in range(8):
        b, half = core // 2, core % 2
        own = x[b, half * SI:(half + 1) * SI]
        other = x[b, (1 - half) * SI:(2 - half) * SI]
        xperm = np.concatenate([own, other], axis=0)
        m = dict(shared)
        m["xn"] = np.ascontiguousarray(_f8(x[b]))
        m["xt"] = np.ascontiguousarray(_f8(xperm.T))
        m["xres"] = np.ascontiguousarray(own + cvec[None, :])
        in_maps.append(m)
    return trivial, in_maps


def kernel(**inputs):
    from concourse.bass_utils import run_bass_kernel_spmd

    trivial, in_maps = _prep_inputs(inputs)
    nc = _get_nc(trivial)
    res = run_bass_kernel_spmd(nc, in_maps, core_ids=list(range(8)))
    out = np.empty((4, S, D), np.float32)
    for core in range(8):
        b, half = core // 2, core % 2
        out[b, half * SI:(half + 1) * SI] = res.results[core]["out"]
    return out


# revision 44
# speedup vs baseline: 1.1534x; 1.1534x over previous
"""Trainium2 Bass kernel for nn_EncoderOnlyBlock (4-head full-dim encoder block).

Sharding: data-parallel, 8 cores = (batch b, seq-half). Each core computes
its 1024 query tokens end-to-end for all 4 heads. K projections are computed
for OWN tokens only and exchanged with the sibling core via a paired
AllGather (DRAM bounce buffers), pipelined for all 4 heads upfront so the
collectives hide under the Q/score chains; kt columns and the x rows feeding
the A@V chains both use the gathered batch-natural token order, keeping the
program fully symmetric across cores.

All attention/projection matmuls run in fp8e4 with DoubleRow perf mode
(2 fp8 MACs/PE-cell/cycle; lhsT/rhs sliced [128, 2, n] over contraction
pairs). Scores are computed TRANSPOSED (S^T = K^T^T Q^T per sj-block) so
exp(S^T) feeds the A@V ("M") chains directly with no PE transposes of the
attention matrix. Softmax normalization is deferred: E = exp(S/sqrt(D)) is
used unnormalized; per-query reciprocal row-sums (ones-vector DR matmuls
over E^T + tiny PE transposes into [si-partition] layout) scale each head's
contribution at the very end. Wv and W1 are fused on the host
(proj_h = A_h x (Wv_h W1_h)), eliminating the Wv chains entirely.

Per-core math (fp8 DR matmuls, fp32 PSUM accumulation):
  K^T_h = (16Wk_h)^T @ x^T / 16          [e, sj]  (bk dropped: softmax-inv.)
  Q^T_h = (16Wq_h)^T @ x^T[:, :1024]/16 + bq      [e, si]
  S^T   = K^T^T Q^T  (per sj-block)      [sj, si]
  E^T   = exp(S^T / 32)  fp8, unnormalized
  rs    = ones^T @ E^T;  rec = 1/(512*rs)         [si(part), head, si-tile]
  M     = x^T @ E^T / 4                  [d, si]  (A@V reassociation)
  proj += rec_h * (M^T @ (2048 Wv_h W1_h))   [si, f]
  cvec  = b1 + sum_h bv_h@W1_h folded into xres on host (A rows sum to 1).
  u1    = xres' + proj;  LN1 -> y (bf16);  z = y@W2 + b2;  LN2(y+z) -> out
LN means/vars via sum & sum-of-squares accumulators; g1/be1/g2/be2 applied
only when not exactly ones/zeros (checked on host). LN1 stats and the first
y transposes are interleaved into head-3's proj loop (lagged) so the
strict-FIFO engine queues pipeline across si-tiles; the z-chains + LN2 run
as a dense PE tail right after. fp8 scale bookkeeping keeps every
tensor within TRN-e4m3 range (max 240): weights x16/x2048 on host, M /4,
with the inverse folded into the activation scales and rec.
"""

import numpy as np
import ml_dtypes

BF = ml_dtypes.bfloat16
F8 = ml_dtypes.float8_e4m3  # TRN-style e4m3: max 240
P = 128
D = 1024
S = 2048
SI = 1024
H = 4
ET = D // P       # 8 e/d/f 128-blocks
SJT = S // P      # 16 sj 128-blocks
SIT = SI // P     # 8 si 128-blocks
SCALE = 1.0 / 32.0  # 1/sqrt(D)
EPS = 1e-5

_CACHE = {}


def _emit(nc, tc, A, trivial_gbe):
    """Emit the per-core program. A: dict name -> dram AP."""
    from contextlib import ExitStack

    import concourse.bass as bass
    import concourse.mybir as mybir
    from concourse.masks import make_identity

    f32 = mybir.dt.float32
    bf16 = mybir.dt.bfloat16
    f8 = mybir.dt.float8e4
    Act = mybir.ActivationFunctionType
    Alu = mybir.AluOpType
    DR = mybir.MatmulPerfMode.DoubleRow

    def dr_chain(ps, lhsT_of, rhs_of, npairs):
        """Emit a DoubleRow accumulation chain of `npairs` k-pair matmuls."""
        for kp in range(npairs):
            nc.tensor.matmul(
                ps,
                lhsT=lhsT_of(kp),
                rhs=rhs_of(kp),
                start=(kp == 0), stop=(kp == npairs - 1),
                perf_mode=DR,
            )

    with ExitStack() as ctx:
        consts = ctx.enter_context(tc.tile_pool(name="consts", bufs=1))
        psA = ctx.enter_context(tc.tile_pool(name="psA", bufs=3, space="PSUM"))
        psB = ctx.enter_context(tc.tile_pool(name="psB", bufs=1, space="PSUM"))
        proj_pool = ctx.enter_context(tc.tile_pool(name="pj", bufs=1))
        # LN-phase inputs whose DMAs we start mid-head-loop
        xr_pool = ctx.enter_context(tc.tile_pool(name="xr", bufs=2))
        w2_pool = ctx.enter_context(tc.tile_pool(name="w2", bufs=ET))
        # LN1 (phase A) pools — phase A is emitted inside head-3's proj loop
        u_pool = ctx.enter_context(tc.tile_pool(name="up", bufs=2))
        sq_pool = ctx.enter_context(tc.tile_pool(name="sq", bufs=1))
        st_pool = ctx.enter_context(tc.tile_pool(name="st", bufs=8))
        lnp = ctx.enter_context(tc.tile_pool(name="lnp", bufs=1))
        yt_pool = ctx.enter_context(tc.tile_pool(name="yt", bufs=2))
        ot_pool = ctx.enter_context(tc.tile_pool(name="ot", bufs=1))

        ident = consts.tile([P, P], bf16, tag="ident")
        make_identity(nc, ident[:])
        bqr_sb = consts.tile([P, H * ET], f32, tag="bqr")
        nc.sync.dma_start(out=bqr_sb[:], in_=A["bqr"][:])
        buv_sb = consts.tile([1, D], bf16, tag="buv")
        nc.sync.dma_start(out=buv_sb[:], in_=A["buv"][:])
        ones_sb = consts.tile([1, P], bf16, tag="ones")
        nc.vector.memset(ones_sb[:], 1.0)
        ones8_sb = consts.tile([P, 2, 16], f8, tag="ones8")
        nc.vector.memset(ones8_sb[:], 1.0)
        eps_sb = consts.tile([P, 1], f32, tag="eps")
        nc.vector.memset(eps_sb[:], EPS)
        # per-head reciprocal row-sums, [si(partition), head, si-tile]
        rsT_sb = consts.tile([P, H, SIT], f32, tag="rsT")
        if not trivial_gbe:
            gbe_sb = lnp.tile([P, 4, D], f32, tag="gbe")
            gbe_bc = bass.AP(
                tensor=A["gbe"].tensor, offset=A["gbe"].offset,
                ap=[[0, P], A["gbe"].ap[0], A["gbe"].ap[1]],
            )
            nc.gpsimd.dma_start(out=gbe_sb[:], in_=gbe_bc)
        yb_sb = lnp.tile([P, SIT, D], bf16, tag="yb")

        def ln_stats(src, rsum):
            """-> (mu, rstd) [P,1] tiles from src [P,D] + its row-sum."""
            sq = sq_pool.tile([P, D], f32, tag="sq")
            sumsq = st_pool.tile([P, 1], f32, tag="sumsq")
            nc.scalar.activation(out=sq[:], in_=src, func=Act.Square,
                                 accum_out=sumsq[:])
            mu = st_pool.tile([P, 1], f32, tag="mu")
            nc.scalar.mul(mu[:], rsum, 1.0 / D)
            # (rsum*mu - sumsq) = -D*var;  std = sqrt(-1/D * that + eps)
            nv = st_pool.tile([P, 1], f32, tag="nv")
            nc.vector.scalar_tensor_tensor(
                out=nv[:], in0=rsum, scalar=mu[:], in1=sumsq[:],
                op0=Alu.mult, op1=Alu.subtract,
            )
            rstd = st_pool.tile([P, 1], f32, tag="rstd")
            nc.scalar.activation(out=rstd[:], in_=nv[:], func=Act.Sqrt,
                                 scale=-1.0 / D, bias=eps_sb[:])
            nc.vector.reciprocal(rstd[:], rstd[:])
            return mu, rstd

        xr_tiles = [None] * SIT

        def emit_phase_a(t):
            """u1 = x + proj -> LN1 stats -> y (bf16) for si-tile t."""
            u1 = u_pool.tile([P, D], f32, tag="u", name=f"u1_{t}")
            rs1 = st_pool.tile([P, 1], f32, tag="rs")
            nc.vector.scalar_tensor_tensor(
                out=u1[:], in0=xr_tiles[t][:], scalar=1.0,
                in1=proj_sb[:, t, :], op0=Alu.mult, op1=Alu.add,
                accum_out=rs1[:],
            )
            mu1, rstd1 = ln_stats(u1[:], rs1[:])
            yt_t = yb_sb[:, t, :]
            nc.vector.tensor_scalar(
                yt_t, u1[:], scalar1=mu1[:], scalar2=rstd1[:],
                op0=Alu.subtract, op1=Alu.mult,
            )
            if not trivial_gbe:
                nc.gpsimd.tensor_mul(yt_t, yt_t, gbe_sb[:, 0, :])
                nc.gpsimd.tensor_add(yt_t, yt_t, gbe_sb[:, 1, :])

        yt_tiles = [None] * SIT
        w2_tiles = []

        def emit_transpose(t):
            yt_tile = yt_pool.tile([P, ET, P], bf16, tag="yt")
            yt_tiles[t] = yt_tile
            pb = psB.tile([P, 1024], bf16, tag="psB")
            for fb in range(ET):
                nc.tensor.transpose(
                    pb[:, fb * P:(fb + 1) * P],
                    yb_sb[:, t, fb * P:(fb + 1) * P],
                    ident[:],
                )
            nc.vector.tensor_copy(
                yt_tile[:], pb[:].rearrange("p (f c) -> p f c", c=P)
            )

        def emit_zchain(t):
            ps = psA.tile([P, 1024], f32, tag="psA")
            for nb in range(2):
                for kc in range(ET):
                    nc.tensor.matmul(
                        ps[:, nb * 512:(nb + 1) * 512],
                        lhsT=yt_tiles[t][:, kc, :],
                        rhs=w2_tiles[kc][:, nb * 512:(nb + 1) * 512],
                        start=(kc == 0), stop=False,
                    )
                nc.tensor.matmul(
                    ps[:, nb * 512:(nb + 1) * 512],
                    lhsT=ones_sb[:, :],
                    rhs=buv_sb[:, nb * 512:(nb + 1) * 512],
                    start=False, stop=True,
                )
            # u2 = y + z + bu; LN2; out
            u2 = u_pool.tile([P, D], f32, tag="u", name=f"u2_{t}")
            rs2 = st_pool.tile([P, 1], f32, tag="rs")
            nc.vector.scalar_tensor_tensor(
                out=u2[:], in0=yb_sb[:, t, :], scalar=1.0,
                in1=ps[:], op0=Alu.mult, op1=Alu.add,
                accum_out=rs2[:],
            )
            mu2, rstd2 = ln_stats(u2[:], rs2[:])
            ot = ot_pool.tile([P, D], f32, tag="ot")
            nc.vector.tensor_scalar(
                ot[:], u2[:], scalar1=mu2[:], scalar2=rstd2[:],
                op0=Alu.subtract, op1=Alu.mult,
            )
            if not trivial_gbe:
                nc.gpsimd.tensor_mul(ot[:], ot[:], gbe_sb[:, 2, :])
                nc.gpsimd.tensor_add(ot[:], ot[:], gbe_sb[:, 3, :])
            nc.sync.dma_start(out=A["out"][t * P:(t + 1) * P, :], in_=ot[:])

        head_ctx = ExitStack()
        xpool = head_ctx.enter_context(tc.tile_pool(name="xp", bufs=1))
        wqkv_pool = head_ctx.enter_context(tc.tile_pool(name="wqkv", bufs=9))
        w1_pool = head_ctx.enter_context(tc.tile_pool(name="w1", bufs=2))
        qt_pool = head_ctx.enter_context(tc.tile_pool(name="qt", bufs=2))
        kt_pool = head_ctx.enter_context(tc.tile_pool(name="kt", bufs=1))
        kown_pool = head_ctx.enter_context(tc.tile_pool(name="kown", bufs=2))
        kdram = head_ctx.enter_context(
            tc.tile_pool(name="kdram", bufs=8, space="DRAM"))
        et_pool = head_ctx.enter_context(tc.tile_pool(name="et", bufs=1))
        m_pool = head_ctx.enter_context(tc.tile_pool(name="m", bufs=1))
        rsb_pool = head_ctx.enter_context(tc.tile_pool(name="rsb", bufs=1))

        # head-0 K weights first (tiny, gate the very first chains), then x^T
        # half-chunks hs-major so the hs=0 K-chains can start after 1MB; x
        # natural after head-0's weights are queued
        wk0_tiles = []
        xt_half = [xpool.tile([P, ET, SI], f8, tag="xth0", name="xth0")]
        for c in range(ET):
            wk_c = wqkv_pool.tile([P, ET, P], f8, tag="wqkv")
            nc.sync.dma_start(out=wk_c[:], in_=A["wkb"][0, c])
            wk0_tiles.append(wk_c)
            nc.sync.dma_start(
                out=xt_half[0][:, c, :],
                in_=A["xt"][c * P:(c + 1) * P, 0:1024],
            )

        xn_sb = xpool.tile([P, SJT, D], f8, tag="xn")

        proj_sb = proj_pool.tile([P, SIT, D], bf16, tag="proj")

        # ---- K^T for OWN tokens only, all 4 heads upfront; sibling half
        # arrives via a paired AllGather (batch-natural column order)
        kgath_tiles = []
        qt_tiles = {}

        def emit_kown(h):
            if h == 0:
                wk_tiles = wk0_tiles
            else:
                wk_tiles = []
                for c in range(ET):
                    wk_c = wqkv_pool.tile([P, ET, P], f8, tag="wqkv")
                    nc.sync.dma_start(out=wk_c[:], in_=A["wkb"][h, c])
                    wk_tiles.append(wk_c)
            kown = kown_pool.tile([P, ET, SI], f8, tag="kown")
            for c in range(ET):
                wk_c = wk_tiles[c]
                ps = psA.tile([P, 1024], f32, tag="psA")
                for nb in range(2):
                    dr_chain(
                        ps[:, nb * 512:(nb + 1) * 512],
                        lambda kp, wk_c=wk_c: wk_c[:, 2 * kp:2 * kp + 2, :],
                        lambda kp, nb=nb: xt_half[0][
                            :, 2 * kp:2 * kp + 2, nb * 512:(nb + 1) * 512],
                        ET // 2,
                    )
                nc.vector.tensor_scalar_mul(kown[:, c, :], ps[:], 1.0 / 16.0)
            kstage = kdram.tile([ET, P, SI], f8, tag="kstage")
            stage_qs = [nc.scalar, nc.sync]
            for c in range(ET):
                for hf in range(2):
                    stage_qs[(2 * c + hf) % 2].dma_start(
                        out=kstage[c, :, hf * 512:(hf + 1) * 512],
                        in_=kown[:, c, hf * 512:(hf + 1) * 512],
                    )
            kgath = kdram.tile([2, ET, P, SI], f8, tag="kgath")
            nc.gpsimd.collective_compute(
                "AllGather", Alu.bypass,
                replica_groups=[[0, 1], [2, 3], [4, 5], [6, 7]],
                ins=[kstage.opt()], outs=[kgath.opt()],
            )
            kgath_tiles.append(kgath)

        def emit_q(h):
            qt_sb = qt_pool.tile([P, ET, SI], f8, tag="qt")
            qt_tiles[h] = qt_sb
            for c in range(ET):
                wq_c = wqkv_pool.tile([P, ET, P], f8, tag="wqkv")
                nc.sync.dma_start(out=wq_c[:], in_=A["wqb"][h, c])
                ps = psA.tile([P, 1024], f32, tag="psA")
                for nb in range(2):
                    dr_chain(
                        ps[:, nb * 512:(nb + 1) * 512],
                        lambda kp: wq_c[:, 2 * kp:2 * kp + 2, :],
                        lambda kp, nb=nb: xt_half[0][
                            :, 2 * kp:2 * kp + 2, nb * 512:(nb + 1) * 512],
                        ET // 2,
                    )
                nc.scalar.activation(
                    out=qt_sb[:, c, :], in_=ps[:], func=Act.Identity,
                    scale=1.0 / 16.0,
                    bias=bqr_sb[:, h * ET + c:h * ET + c + 1],
                )

        for h in range(H):
            emit_kown(h)

        for h in range(H):
            # ---- kt readback: [e, sj] over all 2048 gathered tokens
            kt_sb = kt_pool.tile([P, ET, S], f8, tag="kt")
            kgath = kgath_tiles[h]
            rb_qs = [nc.gpsimd, nc.sync]
            for c in range(ET):
                for r in range(2):
                    rb_qs[c % 2].dma_start(
                        out=kt_sb[:, c, r * 1024:(r + 1) * 1024],
                        in_=kgath[r, c],
                    )

            # ---- Q^T emission hoisted one head ahead (PE runway for the
            # AllGather of head h to complete)
            if h == 0:
                emit_q(0)
            if h + 1 < H:
                emit_q(h + 1)
            qt_sb = qt_tiles[h]

            if h == 0:
                for j in range(SJT):
                    nc.sync.dma_start(out=xn_sb[:, j, :], in_=A["xn"][j * P:(j + 1) * P, :])
            if h == 2:
                # stage the LN-phase inputs; xr via the idle gpsimd queue so a
                # pool-slot wait can't stall head-3 weight DMAs on sync
                for t in range(SIT):
                    xr = xr_pool.tile([P, D], f32, tag="xr", name=f"xr{t}")
                    nc.gpsimd.dma_start(out=xr[:], in_=A["xres"][t * P:(t + 1) * P, :])
                    xr_tiles[t] = xr
                for kc in range(ET):
                    w2_kc = w2_pool.tile([P, D], bf16, tag="w2", name=f"w2{kc}")
                    nc.sync.dma_start(out=w2_kc[:], in_=A["w2"][kc * P:(kc + 1) * P, :])
                    w2_tiles.append(w2_kc)

            # ---- S^T = K^T^T Q^T per sj-block; E^T = exp(S^T/32) fp8
            et_sb = et_pool.tile([P, SJT, SI], f8, tag="et")
            for j in range(SJT):
                ps = psA.tile([P, 1024], f32, tag="psA")
                for half in range(2):
                    dr_chain(
                        ps[:, half * 512:(half + 1) * 512],
                        lambda kp, j=j: kt_sb[:, 2 * kp:2 * kp + 2, j * P:(j + 1) * P],
                        lambda kp, half=half: qt_sb[
                            :, 2 * kp:2 * kp + 2, half * 512:(half + 1) * 512],
                        ET // 2,
                    )
                nc.scalar.activation(
                    out=et_sb[:, j, :], in_=ps[:], func=Act.Exp, scale=SCALE,
                )

            # ---- row-sums: rs[si] = sum_j E^T[j, si]; rec = 1/(8*rs) [si-part]
            rs_ps = psB.tile([1, 1024], f32, tag="psB")
            for jp in range(SJT // 2):
                for half in range(2):
                    nc.tensor.matmul(
                        rs_ps[0:1, half * 512:(half + 1) * 512],
                        lhsT=ones8_sb[:, 0:2, 0:1],
                        rhs=et_sb[:, 2 * jp:2 * jp + 2, half * 512:(half + 1) * 512],
                        start=(jp == 0), stop=(jp == SJT // 2 - 1),
                        perf_mode=DR,
                    )
            rs_bf = rsb_pool.tile([1, 1024], bf16, tag="rsb")
            nc.vector.tensor_copy(rs_bf[:], rs_ps[:])

            # ---- M = x^T @ E^T : [d, si]   (m = psum/4, fp8); the rec
            # transposes are emitted after M dc=0 so the PE doesn't wait on
            # the rs_bf copy (psB bufs=1)
            m_sb = m_pool.tile([P, ET, SI], f8, tag="m")
            for dc in range(ET):
                ps = psA.tile([P, 1024], f32, tag="psA")
                for half in range(2):
                    dr_chain(
                        ps[:, half * 512:(half + 1) * 512],
                        lambda jp, dc=dc: xn_sb[:, 2 * jp:2 * jp + 2, dc * P:(dc + 1) * P],
                        lambda jp, half=half: et_sb[
                            :, 2 * jp:2 * jp + 2, half * 512:(half + 1) * 512],
                        SJT // 2,
                    )
                nc.vector.tensor_scalar_mul(m_sb[:, dc, :], ps[:], 0.25)
                if dc == 0:
                    pb = psB.tile([P, 1024], bf16, tag="psB")
                    for t in range(SIT):
                        nc.tensor.transpose(
                            pb[:, 4 * t:4 * t + 1],
                            rs_bf[0:1, t * P:(t + 1) * P],
                            ident[0:1, 0:1],
                        )
                    pb_str = pb[:, 0:4 * SIT].rearrange("p (t c) -> p t c", c=4)[:, :, 0:1]
                    nc.vector.tensor_scalar_mul(rsT_sb[:, h, :], pb_str, 512.0)
                    nc.vector.reciprocal(rsT_sb[:, h, :], rsT_sb[:, h, :])

            # ---- proj += rec_h * (M_h^T @ (Wv_h W1_h))  (psum = 512*headU@W1)
            w1_h = w1_pool.tile([P, ET, D], f8, tag="w1")
            nc.sync.dma_start(out=w1_h[:], in_=A["w1"][h])
            for t in range(SIT):
                ps = psA.tile([P, 1024], f32, tag="psA")
                for nb in range(2):
                    dr_chain(
                        ps[:, nb * 512:(nb + 1) * 512],
                        lambda ep, t=t: m_sb[:, 2 * ep:2 * ep + 2, t * P:(t + 1) * P],
                        lambda ep, nb=nb: w1_h[
                            :, 2 * ep:2 * ep + 2, nb * 512:(nb + 1) * 512],
                        ET // 2,
                    )
                if h == 0:
                    nc.vector.tensor_scalar_mul(
                        proj_sb[:, t, :], ps[:], rsT_sb[:, 0, t:t + 1]
                    )
                else:
                    nc.vector.scalar_tensor_tensor(
                        out=proj_sb[:, t, :], in0=ps[:],
                        scalar=rsT_sb[:, h, t:t + 1],
                        in1=proj_sb[:, t, :], op0=Alu.mult, op1=Alu.add,
                    )
                if h == H - 1:
                    emit_phase_a(t)
                    if t >= 4:
                        emit_transpose(t - 4)

        head_ctx.close()

        # ====== FFN2 -> LN2 tail: remaining transposes, then dense z-chains
        emit_transpose(SIT - 4)
        emit_transpose(SIT - 3)
        emit_zchain(0)
        emit_transpose(SIT - 2)
        emit_zchain(1)
        emit_transpose(SIT - 1)
        for t in range(2, SIT):
            emit_zchain(t)


def _build(trivial_gbe):
    import concourse.bass as bass
    import concourse.mybir as mybir
    import concourse.tile as tile
    from concourse import bacc

    f32 = mybir.dt.float32
    bf16 = mybir.dt.bfloat16
    f8 = mybir.dt.float8e4

    nc = bacc.Bacc("TRN2", target_bir_lowering=False, debug=False, num_devices=8)
    A = {}

    def din(name, shape, dt):
        A[name] = nc.dram_tensor(name, shape, dt, kind="ExternalInput").ap()

    din("xt", [D, S], f8)
    din("xn", [S, D], f8)
    din("xres", [SI, D], f32)
    din("wqb", [H, ET, P, ET, P], f8)
    din("wkb", [H, ET, P, ET, P], f8)
    din("w1", [H, P, ET, D], f8)
    din("w2", [D, D], bf16)
    din("bqr", [P, H * ET], f32)
    din("buv", [1, D], bf16)
    if not trivial_gbe:
        din("gbe", [4, D], f32)
    A["out"] = nc.dram_tensor("out", [SI, D], f32, kind="ExternalOutput").ap()

    with tile.TileContext(nc) as tc:
        _emit(nc, tc, A, trivial_gbe)
    nc.compile()
    return nc


def _get_nc(trivial_gbe=True):
    key = ("nc", trivial_gbe)
    if key not in _CACHE:
        _CACHE[key] = _build(trivial_gbe)
    return _CACHE[key]


def _f8(a):
    return np.clip(a, -240.0, 240.0).astype(F8)


def _prep_inputs(inputs):
    x = np.ascontiguousarray(inputs["embedding_matrix"], dtype=np.float32)
    Wq = np.asarray(inputs["Wq"], np.float32)
    bq = np.asarray(inputs["bq"], np.float32)
    Wv = np.asarray(inputs["Wv"], np.float32)
    bv = np.asarray(inputs["bv"], np.float32)
    Wk = np.asarray(inputs["Wk"], np.float32)
    W1 = np.asarray(inputs["W1"], np.float32)
    b1 = np.asarray(inputs["b1"], np.float32)
    W2 = np.asarray(inputs["W2"], np.float32)
    b2 = np.asarray(inputs["b2"], np.float32)
    g1 = np.asarray(inputs["g1"], np.float32)
    be1 = np.asarray(inputs["be1"], np.float32)
    g2 = np.asarray(inputs["g2"], np.float32)
    be2 = np.asarray(inputs["be2"], np.float32)

    trivial = (
        np.array_equal(g1, np.ones(D, np.float32))
        and np.array_equal(g2, np.ones(D, np.float32))
        and np.array_equal(be1, np.zeros(D, np.float32))
        and np.array_equal(be2, np.zeros(D, np.float32))
    )

    def pack_w(W, s):  # [H, D, D] -> [H, ET(c), P, ET(kc), P] lhsT blocks
        return np.ascontiguousarray(
            _f8((W * s).reshape(H, ET, P, ET, P).transpose(0, 3, 2, 1, 4))
        )

    wqb = pack_w(Wq, 16.0)
    wkb = pack_w(Wk, 16.0)
    # Wv@W1 fused on host: proj_h = A_h x (Wv_h W1_h).
    # [H, D, D] -> [H, P, ET, D] so one DMA per head gives [P, ET, D]
    wvw1 = np.stack([Wv[h] @ W1[h * D:(h + 1) * D] for h in range(H)])
    w1b = np.ascontiguousarray(
        _f8((wvw1 * 2048.0).reshape(H, ET, P, D).transpose(0, 2, 1, 3))
    )
    w2b = np.ascontiguousarray(W2.astype(BF))
    # bq rearranged so bias for (h, e-block c) is column h*ET+c: [P, H*ET]
    bqr = np.ascontiguousarray(bq.reshape(H, ET, P).transpose(2, 0, 1).reshape(P, H * ET))
    cvec = (b1 + sum(bv[h] @ W1[h * D:(h + 1) * D] for h in range(H)))
    buv = np.ascontiguousarray(b2.reshape(1, D).astype(BF))

    shared = {
        "wqb": wqb, "wkb": wkb, "w1": w1b, "w2": w2b,
        "bqr": bqr, "buv": buv,
    }
    if not trivial:
        shared["gbe"] = np.ascontiguousarray(np.stack([g1, be1, g2, be2]))
    in_maps = []
    for core in range(8):
        b, half = core // 2, core % 2
        own = x[b, half * SI:(half + 1) * SI]
        other = x[b, (1 - half) * SI:(2 - half) * SI]
        xperm = np.concatenate([own, other], axis=0)
        m = dict(shared)
        m["xn"] = np.ascontiguousarray(_f8(x[b]))
        m["xt"] = np.ascontiguousarray(_f8(xperm.T))
        m["xres"] = np.ascontiguousarray(own + cvec[None, :])
        in_maps.append(m)
    return trivial, in_maps


def kernel(**inputs):
    from concourse.bass_utils import run_bass_kernel_spmd

    trivial, in_maps = _prep_inputs(inputs)
    nc = _get_nc(trivial)
    res = run_bass_kernel_spmd(nc, in_maps, core_ids=list(range(8)))
    out = np.empty((4, S, D), np.float32)
    for core in range(8):
        b, half = core // 2, core % 2
        out[b, half * SI:(half + 1) * SI] = res.results[core]["out"]
    return out
